# revision 43
# baseline (speedup 1.0000x reference)
"""GNN message-passing kernel for 8 Trainium2 NeuronCores (Bass/Tile).

Takes FULL inputs, shards nodes across 8 cores internally, runs the
4-layer GNN (dense -> spmm -> spmm -> dense).

v3 design:
- Gather tables (g1/g2) stored as fp8e4m3, but the DRAM tensors are
  typed f32 ([rows, 64]): AllGather cost scales with ELEMENT count
  (2048-element CCE slices), so f32 typing makes the collectives 4x
  cheaper than fp8 typing for the same bytes. Gather/compute APs
  bitcast to fp8; the PE matmul takes fp8 rhs with bf16 lhsT.
- 4 column sub-tables (int16 gather indices), AllGathers fired as soon
  as each shard is written.
- Row-range-major spmm: ranges == sub-table block ranges. Per range:
  4 sub passes (gather + matmul + f32 SBUF accumulate), then the
  range's blocks close (relu + next-layer matmul + shard write), so
  g2's AllGathers fire progressively from ~15% into spmm1.
- Gather calls of 12 chunks rotate over 4 SWDGE queues; block-aligned
  chunks make every chunk a single (chunk, block) matmul. Uniform -1
  idx tails with a matching num_idxs_reg skip padded work safely.
- Closures drain a few per gather call to avoid end-of-phase bursts;
  pmat tile DMAs alternate sync/scalar HWDGE queues.
"""

import math
from contextlib import ExitStack
from dataclasses import dataclass

import ml_dtypes
import numpy as np

import concourse.bass as bass
import concourse.mybir as mybir
import concourse.tile as tile
from concourse import bacc
from concourse.bass_utils import run_bass_kernel_spmd
from concourse.masks import make_identity

BF16 = ml_dtypes.bfloat16
FP8 = ml_dtypes.float8_e4m3
AF = mybir.ActivationFunctionType

CALL_CHUNKS = 16  # chunks per dma_gather call (128 edges per chunk)


@dataclass(frozen=True)
class Cfg:
    n_nodes: int = 50000
    n_edges: int = 800000
    in_dim: int = 512
    h1: int = 512
    h2: int = 256
    out_dim: int = 128
    n_cores: int = 8
    sub_blocks: tuple = (7, 14, 14, 14)  # col row-blocks per sub-table

    @property
    def nodes_per_core(self):
        return math.ceil(self.n_nodes / self.n_cores)

    @property
    def npad(self):
        return math.ceil(self.nodes_per_core / 128) * 128

    @property
    def nblocks(self):
        return self.npad // 128

    @property
    def nsub(self):
        return len(self.sub_blocks)

    @property
    def sub_cumblk(self):
        c = [0]
        for nb_ in self.sub_blocks:
            c.append(c[-1] + nb_)
        return c

    @property
    def srows(self):
        return [nb_ * 128 for nb_ in self.sub_blocks]

    @property
    def stot(self):
        return [r * self.n_cores for r in self.srows]


FULL = Cfg()


# ---------------------------------------------------------------- host prep


def edge_structure(cfg: Cfg, edge_row, edge_col, edge_weight):
    """Per-core streams bucketed by (row-range R, col-sub h).

    Within a stream, each row-block's edges occupy chunk-aligned slots
    (padded to the max count over cores), so every 128-edge chunk maps
    to exactly one (chunk, block) matmul, uniformly across cores.

    meta['streams'][(R, h)] = dict(off16, ncalls, tail_idxs, mm_call)
      mm_call[k] = list of (j, b, col) for call k (j stream-chunk idx,
      col = global pmat column).
    meta['chunks_hb'][h][b], meta['totmm'], meta['idxcols']
    per_core[c] = dict(idx=[128, idxcols] int16, pmat=[128, totmm*128])
    """
    nc_, nb = cfg.n_cores, cfg.nblocks
    npc = cfg.nodes_per_core
    NS = cfg.nsub
    CUM = cfg.sub_cumblk
    assert all(t <= 32767 for t in cfg.stot)
    bounds = np.array([b * 128 for b in CUM])
    srows = np.array(cfg.srows)

    core_of = edge_row // npc
    lr_all = edge_row - core_of * npc
    ccore = edge_col // npc
    clocal = edge_col - ccore * npc
    sub_all = np.searchsorted(bounds, clocal, side="right") - 1
    cl_all = ccore * srows[sub_all] + (clocal - bounds[sub_all])

    per = {}
    counts = np.zeros((nc_, NS, nb), np.int64)
    for c in range(nc_):
        m = core_of == c
        lr, cl, hf, w = lr_all[m], cl_all[m], sub_all[m], edge_weight[m]
        # sort by (sub, row-block, COLUMN): ascending gather addresses
        # within each block span turn random HBM reads into a mostly
        # monotonic sweep (better bank locality); the P matrix absorbs
        # the arbitrary slot -> row mapping for free
        order = np.lexsort((cl, lr // 128, hf))
        per[c] = (lr[order], cl[order], hf[order], w[order])
        np.add.at(counts[c], (per[c][2], per[c][0] // 128), 1)

    chunks_hb = np.ceil(counts.max(axis=0) / 128.0).astype(np.int64)

    streams = {}
    blk_chunk0 = np.zeros((NS, nb), np.int64)  # chunk offset in stream
    tot16 = 0
    jj = 0  # global pmat column
    for R in range(NS):
        for h in range(NS):
            pos = 0
            for b in range(CUM[R], CUM[R + 1]):
                blk_chunk0[h, b] = pos
                pos += int(chunks_hb[h, b])
            used = pos
            ncalls = int(math.ceil(used / float(CALL_CHUNKS)))
            nch = ncalls * CALL_CHUNKS
            mm_call = [[] for _ in range(ncalls)]
            for b in range(CUM[R], CUM[R + 1]):
                for j in range(
                    int(blk_chunk0[h, b]),
                    int(blk_chunk0[h, b] + chunks_hb[h, b]),
                ):
                    mm_call[j // CALL_CHUNKS].append((j, b, jj))
                    jj += 1
            streams[(R, h)] = dict(
                off16=tot16,
                used=used,
                ncalls=ncalls,
                tail_idxs=(
                    (used - (ncalls - 1) * CALL_CHUNKS) * 128
                    if ncalls
                    else 0
                ),
                mm_call=mm_call,
            )
            tot16 += nch * 8
    totmm = jj

    meta = dict(
        streams=streams,
        chunks_hb=chunks_hb,
        totmm=totmm,
        idxcols=max(tot16, 8),
    )

    per_core = []
    for c in range(nc_):
        lr, cl, hf, w = per[c]
        blk = lr // 128
        idx_flat = np.full(meta["idxcols"] * 16, -1, np.int16)
        pmat = np.zeros((128, totmm * 128), BF16)
        for R in range(NS):
            for h in range(NS):
                st = streams[(R, h)]
                base16 = st["off16"]
                sel = (hf == h) & (blk >= CUM[R]) & (blk < CUM[R + 1])
                e_cl, e_lr, e_w = cl[sel], lr[sel], w[sel]
                eb = e_lr // 128
                bstart = np.searchsorted(eb, np.arange(nb))
                bend = np.searchsorted(eb, np.arange(nb), side="right")
                slot_of = np.zeros(len(e_cl), np.int64)
                for b in range(CUM[R], CUM[R + 1]):
                    s0, s1 = int(bstart[b]), int(bend[b])
                    if s1 > s0:
                        slot_of[s0:s1] = blk_chunk0[h, b] * 128 + np.arange(
                            s1 - s0
                        )
                used = st["used"]
                vals = np.zeros(used * 128, np.int16)
                vals[slot_of] = e_cl.astype(np.int16)
                i_in = np.arange(used * 128)
                idx_flat[(base16 + i_in // 16) * 16 + (i_in % 16)] = vals
                # global col of chunk j in this stream: cols are numbered
                # consecutively in j order within the stream
                col0 = st["mm_call"][0][0][2] if st["mm_call"][0] else 0
                jglob = col0 + slot_of // 128
                pmat[slot_of % 128, jglob * 128 + (e_lr - eb * 128)] = (
                    e_w.astype(BF16)
                )
        idx_mat = idx_flat.reshape(meta["idxcols"], 16).T
        idx_mat = np.tile(idx_mat, (8, 1))
        per_core.append(
            dict(idx=np.ascontiguousarray(idx_mat), pmat=pmat)
        )

    return meta, per_core


def prep_inputs(cfg: Cfg, inputs):
    f = inputs["features"].astype(np.float32)
    meta, per_edge = edge_structure(
        cfg,
        inputs["edge_row"].astype(np.int64),
        inputs["edge_col"].astype(np.int64),
        inputs["edge_weight"].astype(np.float32),
    )
    kin = cfg.in_dim // 128
    k1 = cfg.h1 // 128
    k2 = cfg.h2 // 128

    def wlayout(w, kt):
        K, M = w.shape
        return (
            w.reshape(kt, 128, M).transpose(1, 0, 2).reshape(128, kt * M)
        ).astype(BF16)

    w1 = wlayout(inputs["W_lin1"].astype(np.float32), kin)
    wg1 = wlayout(inputs["W_g1"].astype(np.float32), k1)
    wg2 = wlayout(inputs["W_g2"].astype(np.float32), k2)
    wl2 = wlayout(inputs["W_lin2"].astype(np.float32), k2)
    b1 = inputs["b_lin1"].astype(np.float32).reshape(kin, 128).T.copy()
    bg1b = np.tile(
        inputs["b_g1"].astype(np.float32).reshape(1, cfg.h2), (128, 1)
    )
    bg2b = np.tile(
        inputs["b_g2"].astype(np.float32).reshape(1, cfg.h2), (128, 1)
    )
    bl2 = inputs["b_lin2"].astype(BF16).reshape(1, cfg.out_dim)

    npc, npad = cfg.nodes_per_core, cfg.npad
    in_maps = []
    for c in range(cfg.n_cores):
        lo = c * npc
        hi = min((c + 1) * npc, cfg.n_nodes)
        xc = np.zeros((npad, cfg.in_dim), np.float32)
        xc[: hi - lo] = f[lo:hi]
        NR = math.ceil(npad / 512)
        xt = (
            xc.T.reshape(kin, 128, npad)
            .transpose(1, 0, 2)
            .reshape(128, kin, npad)
        )
        xtp = np.zeros((128, kin, NR * 512), np.float32)
        xtp[:, :, :npad] = xt
        xt = (
            xtp.reshape(128, kin, NR, 512)
            .transpose(0, 2, 1, 3)
            .reshape(128, NR * kin * 512)
        ).astype(BF16)
        in_maps.append(
            {
                "xt": np.ascontiguousarray(xt),
                "w1": w1,
                "wg1": wg1,
                "wg2": wg2,
                "wl2": wl2,
                "b1": b1,
                "bg1b": bg1b,
                "bg2b": bg2b,
                "bl2": bl2,
                "idx": per_edge[c]["idx"],
                "pmat": per_edge[c]["pmat"],
            }
        )
    return meta, in_maps


# ---------------------------------------------------------------- kernel IR


def build(cfg: Cfg, meta):
    nc = bacc.Bacc(
        "TRN2",
        target_bir_lowering=False,
        debug=False,
        num_devices=cfg.n_cores,
        num_swdge_queues=4,
        dynamic_dma_scratch_size=32768,
    )
    bf = mybir.dt.bfloat16
    f32 = mybir.dt.float32
    f8 = mybir.dt.float8e4
    i16 = mybir.dt.int16
    kin = cfg.in_dim // 128
    k1 = cfg.h1 // 128
    k2 = cfg.h2 // 128
    npad, nb, H2, OUT = cfg.npad, cfg.nblocks, cfg.h2, cfg.out_dim
    NS = cfg.nsub
    CUM = cfg.sub_cumblk
    CC = CALL_CHUNKS
    H2w = H2 // 4  # f32 words per table row

    NR = math.ceil(npad / 512)
    xt_d = nc.dram_tensor(
        "xt", [128, NR * kin * 512], bf, kind="ExternalInput"
    ).ap()
    w1_d = nc.dram_tensor("w1", [128, kin * cfg.h1], bf, kind="ExternalInput").ap()
    wg1_d = nc.dram_tensor("wg1", [128, k1 * H2], bf, kind="ExternalInput").ap()
    wg2_d = nc.dram_tensor("wg2", [128, k2 * H2], bf, kind="ExternalInput").ap()
    wl2_d = nc.dram_tensor("wl2", [128, k2 * OUT], bf, kind="ExternalInput").ap()
    b1_d = nc.dram_tensor("b1", [128, kin], f32, kind="ExternalInput").ap()
    bg1b_d = nc.dram_tensor("bg1b", [128, H2], f32, kind="ExternalInput").ap()
    bg2b_d = nc.dram_tensor("bg2b", [128, H2], f32, kind="ExternalInput").ap()
    bl2_d = nc.dram_tensor("bl2", [1, OUT], bf, kind="ExternalInput").ap()
    idx_d = nc.dram_tensor(
        "idx", [128, meta["idxcols"]], i16, kind="ExternalInput"
    ).ap()
    pmat_d = nc.dram_tensor(
        "pmat", [128, meta["totmm"] * 128], bf, kind="ExternalInput"
    ).ap()
    y_d = nc.dram_tensor("y", [npad, OUT], f32, kind="ExternalOutput").ap()

    # tiny dummy collective issued first: absorbs the NEFF-level
    # cross-core barrier / launch skew behind L1 instead of delaying
    # the first real AllGather
    dml = nc.dram_tensor("dml", [1, 16], f32).ap()
    dmt = nc.dram_tensor("dmt", [8, 16], f32, addr_space="Shared").ap()

    # collective tensors typed f32 (4x fewer elements than fp8 typing);
    # gather/compute access them via fp8 bitcast
    g1l = [
        nc.dram_tensor(f"g1l{s}", [cfg.srows[s], H2w], f32).ap()
        for s in range(NS)
    ]
    g2l = [
        nc.dram_tensor(f"g2l{s}", [cfg.srows[s], H2w], f32).ap()
        for s in range(NS)
    ]
    g1t = [
        nc.dram_tensor(
            f"g1t{s}", [cfg.stot[s], H2w], f32, addr_space="Shared"
        ).ap()
        for s in range(NS)
    ]
    g2t = [
        nc.dram_tensor(
            f"g2t{s}", [cfg.stot[s], H2w], f32, addr_space="Shared"
        ).ap()
        for s in range(NS)
    ]

    rg = [list(range(cfg.n_cores))]

    def allgather(local, table):
        nc.gpsimd.collective_compute(
            "AllGather",
            mybir.AluOpType.bypass,
            replica_groups=rg,
            ins=[local[:, :]],
            outs=[table[:, :]],
        )

    def sub_of_block(b):
        for s in range(NS):
            if b < CUM[s + 1]:
                return s
        raise AssertionError(b)

    # first/last sub with edges per block (uniform across cores)
    chunks_hb = meta["chunks_hb"]
    first_sub = {}
    last_sub = {}
    for b in range(nb):
        subs = [h for h in range(NS) if chunks_hb[h][b] > 0]
        if subs:
            first_sub[b] = subs[0]
            last_sub[b] = subs[-1]
    # last stream-chunk of each block within each stream
    last_chunk = {}
    for (R, h), st in meta["streams"].items():
        for mmk in st["mm_call"]:
            for (j, b, col) in mmk:
                last_chunk[(R, h, b)] = j

    qctr = [0]
    dmactr = [0]

    with tile.TileContext(nc) as tc:
        with ExitStack() as top:
            allgather(dml, dmt)
            # per-call gather index counts live in persistent registers so
            # each dma_gather costs one engine-queue slot (no MOVE per call)
            reg_full = nc.gpsimd.compute_val(CC * 128)
            tail_vals = sorted(
                {
                    st["tail_idxs"]
                    for st in meta["streams"].values()
                    if st["ncalls"] and st["tail_idxs"] != CC * 128
                }
            )
            tail_regs = {v: nc.gpsimd.compute_val(v) for v in tail_vals}
            tail_regs[CC * 128] = reg_full

            const = top.enter_context(tc.tile_pool(name="const", bufs=1))
            w1_s = const.tile([128, kin * cfg.h1], bf)
            nc.sync.dma_start(w1_s[:], w1_d[:, :])
            wg1_s = const.tile([128, k1 * H2], bf)
            nc.sync.dma_start(wg1_s[:], wg1_d[:, :])
            wg2_s = const.tile([128, k2 * H2], bf)
            nc.sync.dma_start(wg2_s[:], wg2_d[:, :])
            wl2_s = const.tile([128, k2 * OUT], bf)
            nc.sync.dma_start(wl2_s[:], wl2_d[:, :])
            b1_s = const.tile([128, kin], f32)
            nc.sync.dma_start(b1_s[:], b1_d[:, :])
            bg1b_s = const.tile([128, H2], f32)
            nc.sync.dma_start(bg1b_s[:], bg1b_d[:, :])
            bg2b_s = const.tile([128, H2], f32)
            nc.sync.dma_start(bg2b_s[:], bg2b_d[:, :])
            bl2_s = const.tile([1, OUT], bf)
            nc.sync.dma_start(bl2_s[:], bl2_d[:, :])
            idx_s = const.tile([128, meta["idxcols"]], i16)
            nc.sync.dma_start(idx_s[:], idx_d[:, :])
            ident = const.tile([128, 128], bf)
            make_identity(nc, ident[:])
            ones_t = const.tile([1, 128], bf)
            nc.gpsimd.memset(ones_t[:], 1.0)
            acc = const.tile([128, nb, H2], bf)
            nc.vector.memset(acc[:], 0.0)

            # ---- L1 + L2a interleaved per 512-node range -----------------
            with ExitStack() as px:
                xp = px.enter_context(tc.tile_pool(name="xt", bufs=3))
                psp = px.enter_context(
                    tc.tile_pool(name="ps1", bufs=4, space="PSUM")
                )
                psp2 = px.enter_context(
                    tc.tile_pool(name="ps2", bufs=3, space="PSUM")
                )
                h1p = px.enter_context(tc.tile_pool(name="h1r", bufs=3))
                tp2 = px.enter_context(tc.tile_pool(name="g1t", bufs=3))
                for r in range(NR):
                    a = r * 512
                    nw = min(512, npad - a)
                    xr = xp.tile([128, kin * 512], bf, tag="x")
                    nc.sync.dma_start(
                        xr[:], xt_d[:, r * kin * 512 : (r + 1) * kin * 512]
                    )
                    h1r = h1p.tile([128, k1, 512], bf, tag="h")
                    for f1t in range(k1):
                        ps = psp.tile([128, 512], f32, tag="ps")
                        for kt in range(kin):
                            nc.tensor.matmul(
                                ps[:, :nw],
                                lhsT=w1_s[
                                    :,
                                    kt * cfg.h1
                                    + f1t * 128 : kt * cfg.h1
                                    + f1t * 128
                                    + 128,
                                ],
                                rhs=xr[:, kt * 512 : kt * 512 + nw],
                                start=(kt == 0),
                                stop=(kt == kin - 1),
                            )
                        nc.scalar.activation(
                            h1r[:, f1t, :nw],
                            ps[:, :nw],
                            AF.Sigmoid,
                            bias=b1_s[:, f1t : f1t + 1],
                        )
                    for b in range(a // 128, (a + nw) // 128):
                        off = b * 128 - a
                        ps2 = psp2.tile([128, H2], f32, tag="ps")
                        for kt in range(k1):
                            nc.tensor.matmul(
                                ps2[:],
                                lhsT=h1r[:, kt, off : off + 128],
                                rhs=wg1_s[:, kt * H2 : (kt + 1) * H2],
                                start=(kt == 0),
                                stop=(kt == k1 - 1),
                            )
                        g1tile = tp2.tile([128, H2], f8, tag="g1")
                        nc.vector.tensor_copy(g1tile[:], ps2[:])
                        s = sub_of_block(b)
                        bb = b - CUM[s]
                        nc.sync.dma_start(
                            g1l[s][bb * 128 : (bb + 1) * 128, :],
                            g1tile[:].bitcast(f32),
                        )
                        if b == CUM[s + 1] - 1:
                            allgather(g1l[s], g1t[s])

            # ---- spmm passes, range-major --------------------------------
            with ExitStack() as sx:
                gp = sx.enter_context(tc.tile_pool(name="gath", bufs=14))
                pp = sx.enter_context(tc.tile_pool(name="pmat", bufs=4))
                sp = sx.enter_context(
                    tc.tile_pool(name="psb", bufs=5, space="PSUM")
                )
                tps = sx.enter_context(
                    tc.tile_pool(name="tps", bufs=1, space="PSUM")
                )
                ps34 = sx.enter_context(
                    tc.tile_pool(name="ps34", bufs=2, space="PSUM")
                )
                clp = sx.enter_context(tc.tile_pool(name="clt", bufs=3))

                pending_close = []  # (layer, block) awaiting closure
                pend_ag = []

                def close_block_l1(b):
                    h2t = clp.tile([128, H2], bf, tag="h2")
                    nc.scalar.activation(h2t[:], acc[:, b, :], AF.Relu)
                    pt = tps.tile([128, k2, 128], bf, tag="pt")
                    for kt in range(k2):
                        nc.tensor.transpose(
                            pt[:, kt, :],
                            h2t[:, kt * 128 : (kt + 1) * 128],
                            ident[:],
                        )
                    h2T = clp.tile([128, k2, 128], bf, tag="h2T")
                    nc.vector.tensor_copy(h2T[:], pt[:])
                    ps3 = ps34.tile([128, H2], f32, tag="ps3")
                    for kt in range(k2):
                        nc.tensor.matmul(
                            ps3[:],
                            lhsT=h2T[:, kt, :],
                            rhs=wg2_s[:, kt * H2 : (kt + 1) * H2],
                            start=(kt == 0),
                            stop=(kt == k2 - 1),
                        )
                    g2tile = clp.tile([128, H2], f8, tag="g2")
                    nc.vector.tensor_copy(g2tile[:], ps3[:])
                    s = sub_of_block(b)
                    bb = b - CUM[s]
                    nc.sync.dma_start(
                        g2l[s][bb * 128 : (bb + 1) * 128, :],
                        g2tile[:].bitcast(f32),
                    )
                    if b == CUM[s + 1] - 1:
                        pend_ag.append((s, qctr[0] + 1))

                def close_block_l2(b):
                    h3t = clp.tile([128, H2], bf, tag="h2")
                    nc.scalar.activation(h3t[:], acc[:, b, :], AF.Relu)
                    pt = tps.tile([128, k2, 128], bf, tag="pt")
                    for kt in range(k2):
                        nc.tensor.transpose(
                            pt[:, kt, :],
                            h3t[:, kt * 128 : (kt + 1) * 128],
                            ident[:],
                        )
                    h3T = clp.tile([128, k2, 128], bf, tag="h2T")
                    nc.vector.tensor_copy(h3T[:], pt[:])
                    ps4 = ps34.tile([128, OUT], f32, tag="ps3")
                    for kt in range(k2):
                        nc.tensor.matmul(
                            ps4[:],
                            lhsT=h3T[:, kt, :],
                            rhs=wl2_s[:, kt * OUT : (kt + 1) * OUT],
                            start=(kt == 0),
                            stop=False,
                        )
                    nc.tensor.matmul(
                        ps4[:],
                        lhsT=ones_t[:1, :],
                        rhs=bl2_s[:1, :],
                        start=False,
                        stop=True,
                    )
                    yt = clp.tile([128, OUT], f32, tag="y")
                    nc.vector.tensor_copy(yt[:], ps4[:])
                    nc.sync.dma_start(y_d[b * 128 : (b + 1) * 128, :], yt[:])

                def drain(nclose):
                    # fire deferred AllGathers only once the gather stream
                    # has advanced enough that the shard-write chain has
                    # caught up (the AG's wait head-blocks the gpsimd NX)
                    while pend_ag and qctr[0] >= pend_ag[0][1]:
                        s, _ = pend_ag.pop(0)
                        allgather(g2l[s], g2t[s])
                    for _ in range(nclose):
                        if not pending_close:
                            break
                        lay, b = pending_close.pop(0)
                        (close_block_l1 if lay == 1 else close_block_l2)(b)

                # layer 1: frontload range 0 so g2's first shard closes
                # early (each sub h still entered after AllGather h lands);
                # layer 2: closures have no downstream AGs, so sub-major
                # order maximizes AllGather readiness
                order_l1 = [
                    (0, 0), (1, 0), (0, 1), (0, 2), (1, 1), (0, 3),
                    (2, 0), (1, 2), (2, 1), (1, 3), (3, 0), (2, 2),
                    (3, 1), (2, 3), (3, 2), (3, 3),
                ]
                order_l2 = [
                    (R, h) for h in range(NS) for R in range(NS)
                ]
                assert len(order_l1) == NS * NS

                for layer in (1, 2):
                    tables = g1t if layer == 1 else g2t
                    bias_s = bg1b_s if layer == 1 else bg2b_s
                    psums = {}
                    for (R, h) in (order_l1 if layer == 1 else order_l2):
                        if True:
                            st = meta["streams"][(R, h)]
                            for k in range(st["ncalls"]):
                                drain(3)
                                t = gp.tile([128, CC, H2], f8, tag="g")
                                o16 = st["off16"] + k * CC * 8
                                ns_ch = min(CC, st["used"] - k * CC)
                                nc.gpsimd.dma_gather(
                                    out_ap=t[:, :ns_ch, :],
                                    in_ap=tables[h][:, :].bitcast(f8),
                                    idxs_ap=idx_s[:, o16 : o16 + ns_ch * 8],
                                    num_idxs=ns_ch * 128,
                                    num_idxs_reg=tail_regs[ns_ch * 128],
                                    elem_size=H2,
                                    single_packet=False,
                                    queue_num=qctr[0] % 4,
                                )
                                qctr[0] += 1
                                mmk = st["mm_call"][k]
                                if mmk:
                                    nmm = len(mmk)
                                    c0 = mmk[0][2]
                                    ptile = pp.tile(
                                        [128, nmm * 128], bf, tag="p"
                                    )
                                    eng = (
                                        nc.scalar
                                        if dmactr[0] % 2
                                        else nc.sync
                                    )
                                    dmactr[0] += 1
                                    eng.dma_start(
                                        ptile[:],
                                        pmat_d[
                                            :, c0 * 128 : (c0 + nmm) * 128
                                        ],
                                    )
                                for i, (j, b, col) in enumerate(mmk):
                                    if b not in psums:
                                        psums[b] = sp.tile(
                                            [128, H2],
                                            f32,
                                            tag="ps",
                                            name=f"ps{layer}_{R}_{h}_{b}",
                                        )
                                        started = False
                                    else:
                                        started = True
                                    is_last = j == last_chunk[(R, h, b)]
                                    nc.tensor.matmul(
                                        psums[b][:],
                                        lhsT=ptile[
                                            :, i * 128 : (i + 1) * 128
                                        ],
                                        rhs=t[:, j - k * CC, :],
                                        start=not started,
                                        stop=is_last,
                                    )
                                    if is_last:
                                        ps_b = psums.pop(b)
                                        if first_sub[b] == h:
                                            nc.vector.tensor_add(
                                                acc[:, b, :],
                                                ps_b[:],
                                                bias_s[:],
                                            )
                                        else:
                                            nc.vector.tensor_add(
                                                acc[:, b, :],
                                                acc[:, b, :],
                                                ps_b[:],
                                            )
                                        if last_sub[b] == h:
                                            pending_close.append(
                                                (layer, b)
                                            )
                    # blocks with no edges at all still need closing
                    for b in range(nb):
                        if b not in first_sub:
                            pending_close.append((layer, b))
                # tail: flush remaining closures and AGs (force-fire AGs
                # since qctr no longer advances)
                while pending_close or pend_ag:
                    while pend_ag:
                        s, _ = pend_ag.pop(0)
                        allgather(g2l[s], g2t[s])
                    for _ in range(4):
                        if not pending_close:
                            break
                        lay, b = pending_close.pop(0)
                        (close_block_l1 if lay == 1 else close_block_l2)(b)

    nc.compile()
    return nc


# ---------------------------------------------------------------- driver

_CACHE = {}


def run(inputs, cfg: Cfg = FULL, trace=False, tmpdir=None):
    meta, in_maps = prep_inputs(cfg, inputs)
    key = (cfg, meta["totmm"], meta["idxcols"])
    if key not in _CACHE:
        _CACHE[key] = build(cfg, meta)
    nc = _CACHE[key]
    res = run_bass_kernel_spmd(
        nc,
        in_maps,
        core_ids=list(range(cfg.n_cores)),
        trace=trace,
        tmpdir=tmpdir,
    )
    npc = cfg.nodes_per_core
    out = np.empty((cfg.n_nodes, cfg.out_dim), np.float32)
    for c in range(cfg.n_cores):
        lo = c * npc
        hi = min((c + 1) * npc, cfg.n_nodes)
        out[lo:hi] = res.results[c]["y"][: hi - lo]
    return out, res


def kernel(**inputs) -> np.ndarray:
    out, _ = run(inputs, FULL, trace=False)
    return out
